# revision 9
# baseline (speedup 1.0000x reference)
"""Trainium2 Bass kernel for DPRNN (dropout RNN) — data-parallel over 8 cores.

Model (per batch element b, T=50 steps, I=2, H=20, O=2):
    xp[t] = x[t] @ W_ih.T + b_ih + b_hh
    h[t]  = tanh(xp[t] + h[t-1] @ W_hh.T),  h[-1] = 0
    out[t] = (h[t] * mask[t]) @ W_out.T + b_out

Device strategy (per core, B/8 batch rows):
  - hidden dim lives on SBUF partitions; G=6 independent batch groups are
    packed block-diagonally so matmuls use 120 of 128 partitions.
  - all DRAM<->SBUF traffic is contiguous (>=5KB runs per partition) because
    the host pre-permutes x / mask / out layouts (layout prep only, no FLOPs).
  - per timestep: in-proj matmul + recurrence matmul accumulate in PSUM,
    ACT tanh(+bias) -> h, DVE mask-mul, out-proj matmul into a PSUM tile
    whose partition offset is 32*(t%4) (PE tile_position constraint), then
    one ACT copy(+bias) per chunk and one strided DMA out per 4 timesteps.
"""

import numpy as np

B, T, I, H, O = 65536, 50, 2, 20, 2
NCORES = 8
G = 6                      # batch groups packed along partitions
NC = 1366                  # batch columns per group per core
BCORE = G * NC             # 8196 padded batch rows per core
BPAD = NCORES * BCORE      # 65568
PH, PI, PO = G * H, G * I, G * O   # 120, 12, 12
TS = 4                     # timesteps per out-PSUM supergroup
PSTRIDE = 32               # partition offset per timestep within supergroup
PSO_ROWS = TS * PSTRIDE   # 128 (out-proj writes full 32-row stripes)
NGRP = (T + TS - 1) // TS  # 13 output supergroups (12 full + 1 of 2)
CHUNKS = [(0, 512), (512, 512), (1024, NC - 1024)]  # psum bank-aligned chunks

_CACHE = {}


def _build_module(repeat=1):
    import concourse.bass as bass
    import concourse.bacc as bacc
    import concourse.tile as tile
    from concourse import mybir

    f32 = mybir.dt.float32
    TANH = mybir.ActivationFunctionType.Tanh
    IDENT = mybir.ActivationFunctionType.Identity

    nc = bacc.Bacc("TRN2", target_bir_lowering=False, debug=False,
                   num_devices=NCORES)

    xT = nc.dram_tensor("xT", [T, PI, NC], f32, kind="ExternalInput")
    maskh = nc.dram_tensor("maskh", [T, PH, NC], f32, kind="ExternalInput")
    wih = nc.dram_tensor("wih", [PI, PH], f32, kind="ExternalInput")
    whh = nc.dram_tensor("whh", [PH, PH], f32, kind="ExternalInput")
    wout = nc.dram_tensor("wout", [PH, PSTRIDE], f32, kind="ExternalInput")
    bh = nc.dram_tensor("bh", [PH, 1], f32, kind="ExternalInput")
    bo = nc.dram_tensor("bo", [PSO_ROWS, 1], f32, kind="ExternalInput")
    outh = nc.dram_tensor("outh", [T, PO, NC], f32, kind="ExternalOutput")

    xT_ap, maskh_ap, outh_ap = xT.ap(), maskh.ap(), outh.ap()

    with tile.TileContext(nc) as tc:
        with (
            tc.tile_pool(name="w", bufs=1) as wp,
            tc.tile_pool(name="x", bufs=3) as xp,
            tc.tile_pool(name="mask", bufs=3) as mp,
            tc.tile_pool(name="h", bufs=2) as hp,
            tc.tile_pool(name="rm", bufs=3) as rp,
            tc.tile_pool(name="osb", bufs=2) as op,
            tc.tile_pool(name="psr", bufs=2, space=bass.MemorySpace.PSUM) as pr,
            tc.tile_pool(name="pso", bufs=2, space=bass.MemorySpace.PSUM) as po,
        ):
            w_ih = wp.tile([PI, PH], f32)
            nc.sync.dma_start(w_ih[:], wih.ap())
            w_hh = wp.tile([PH, PH], f32)
            nc.sync.dma_start(w_hh[:], whh.ap())
            w_out = wp.tile([PH, PSTRIDE], f32)
            nc.sync.dma_start(w_out[:], wout.ap())
            b_h = wp.tile([PH, 1], f32)
            nc.sync.dma_start(b_h[:], bh.ap())
            b_o = wp.tile([PSO_ROWS, 1], f32)
            nc.sync.dma_start(b_o[:], bo.ap())

            for rep in range(repeat):
                h_prev = None
                ps_o = None
                for t in range(T):
                    grp, t8 = t // TS, t % TS
                    cur_ts = min(TS, T - grp * TS)
                    orows = cur_ts * PSTRIDE

                    x_t = xp.tile([PI, NC], f32, tag="x")
                    nc.sync.dma_start(x_t[:], xT_ap[t])
                    m_t = mp.tile([PH, NC], f32, tag="mask")
                    nc.sync.dma_start(m_t[:], maskh_ap[t])

                    h_new = hp.tile([PH, NC], f32, tag="h")
                    if t8 == 0:
                        ps_o = [po.tile([orows, 512], f32, tag=f"pso{c}",
                                        name=f"pso{c}_{rep}_{grp}")[:, :n]
                                for c, (s, n) in enumerate(CHUNKS)]

                    for c, (s, n) in enumerate(CHUNKS):
                        ps = pr.tile([PH, 512], f32, tag="psr", name=f"psr_{rep}_{t}_{c}")[:, :n]
                        nc.tensor.matmul(ps[:], w_ih[:], x_t[:, s:s + n],
                                         start=True, stop=(t == 0))
                        if t > 0:
                            nc.tensor.matmul(ps[:], w_hh[:],
                                             h_prev[:, s:s + n],
                                             start=False, stop=True)
                        nc.scalar.activation(h_new[:, s:s + n], ps[:], TANH,
                                             bias=b_h[:])
                        rm = rp.tile([PH, n], f32, tag="rm")
                        nc.vector.tensor_mul(rm[:], h_new[:, s:s + n],
                                             m_t[:, s:s + n])
                        base = t8 * PSTRIDE
                        nc.tensor.matmul(ps_o[c][base:base + PSTRIDE, :],
                                         w_out[:], rm[:],
                                         start=True, stop=True,
                                         tile_position=(0, base))

                    if t8 == cur_ts - 1:
                        o_sb = op.tile([cur_ts * PSTRIDE, NC], f32, tag="osb")
                        for c, (s, n) in enumerate(CHUNKS):
                            nc.scalar.activation(o_sb[:orows, s:s + n],
                                                 ps_o[c][:],
                                                 IDENT, bias=b_o[:orows, :])
                        for k in range(cur_ts):
                            nc.sync.dma_start(
                                outh_ap[grp * TS + k],
                                o_sb[k * PSTRIDE:k * PSTRIDE + PO, :])
                    h_prev = h_new

    nc.compile()
    return nc


def _get_module(repeat=1):
    key = ("nc", repeat)
    if key not in _CACHE:
        _CACHE[key] = _build_module(repeat)
    return _CACHE[key]


def pack_inputs(x, W_ih, W_hh, b_ih, b_hh, W_out, b_out, drop_mask):
    """Host-side shard + layout permute. Returns list of 8 in_maps."""
    x = np.asarray(x, np.float32)
    drop_mask = np.asarray(drop_mask, np.float32)
    W_ih = np.asarray(W_ih, np.float32)
    W_hh = np.asarray(W_hh, np.float32)
    W_out = np.asarray(W_out, np.float32)
    b_ih = np.asarray(b_ih, np.float32)
    b_hh = np.asarray(b_hh, np.float32)
    b_out = np.asarray(b_out, np.float32)

    xpad = np.zeros((BPAD, T, I), np.float32)
    xpad[:B] = x
    mk = np.zeros((BPAD, T, H), np.float32)
    mk[:B] = drop_mask

    # [core, G, NC, T, *] -> [core, T, G, *, NC]
    xr = xpad.reshape(NCORES, G, NC, T, I).transpose(0, 3, 1, 4, 2)
    xT = np.ascontiguousarray(xr).reshape(NCORES, T, PI, NC)
    mr = mk.reshape(NCORES, G, NC, T, H).transpose(0, 3, 1, 4, 2)
    maskh = np.ascontiguousarray(mr).reshape(NCORES, T, PH, NC)

    wih_blk = np.zeros((PI, PH), np.float32)
    whh_blk = np.zeros((PH, PH), np.float32)
    wout_blk = np.zeros((PH, PSTRIDE), np.float32)
    for g in range(G):
        wih_blk[g * I:(g + 1) * I, g * H:(g + 1) * H] = W_ih.T
        whh_blk[g * H:(g + 1) * H, g * H:(g + 1) * H] = W_hh.T
        wout_blk[g * H:(g + 1) * H, g * O:(g + 1) * O] = W_out.T
    bh_v = np.tile(b_ih + b_hh, G).reshape(PH, 1).astype(np.float32)
    bo_v = np.zeros((PSO_ROWS, 1), np.float32)
    for k in range(TS):
        bo_v[k * PSTRIDE:k * PSTRIDE + PO, 0] = np.tile(b_out, G)

    return [{
        "xT": xT[c].copy(),
        "maskh": maskh[c].copy(),
        "wih": wih_blk, "whh": whh_blk, "wout": wout_blk,
        "bh": bh_v, "bo": bo_v,
    } for c in range(NCORES)]


def unpack_output(outh_list):
    """outh_list: 8 arrays [T, PO, NC] -> full [B, T, O]."""
    o = np.stack([np.asarray(a) for a in outh_list])      # [8, T, PO, NC]
    o = o.reshape(NCORES, T, G, O, NC).transpose(0, 2, 4, 1, 3)
    return np.ascontiguousarray(o).reshape(BPAD, T, O)[:B]


def kernel(x, W_ih, W_hh, b_ih, b_hh, W_out, b_out, drop_mask):
    from concourse import bass_utils
    nc = _get_module()
    in_maps = pack_inputs(x, W_ih, W_hh, b_ih, b_hh, W_out, b_out, drop_mask)
    res = bass_utils.run_bass_kernel_spmd(nc, in_maps,
                                          core_ids=list(range(NCORES)))
    return unpack_output([r["outh"] for r in res.results])
